# revision 1
# baseline (speedup 1.0000x reference)
"""MoSARA MoE-routing kernel for 8 Trainium2 NeuronCores.

Math: the reference materializes per-expert delta weights
    delta_W[e] = U_k @ diag(lambda_k[e]) @ V_k,  out = sum_e g[b,e] * x @ (W+delta_W[e]).T
but since softmax gates sum to 1 this collapses to
    out = (x @ W.T + ((x @ V_k.T) * (g @ lambda_k)) @ U_k.T) * (1+v)
with g = softmax_e((x @ U_k @ router_W1) * router_W2[e]).

Host-side preprocessing (all exact, fp32):
  - fold (1+v) into W and U_k rows,
  - precompute u1 = U_k @ router_W1 (rank-1 router),
  - pre-transpose operands so the contraction dim lands on SBUF partitions,
  - cast matmul operands to bf16 (fp32 accumulation in PSUM).

Device per core (data-parallel over B, 512 tokens/core):
  s1 = u1.T @ xT                  (1,512)    router logit scale
  sT = V.T-chunks @ xT            (512,512)  low-rank projection
  logits = W2[e]*s1[b] - m[b]  via one K=2 matmul; m = exact row max
  g = exp(logits); den = ones @ g; gn = g * bcast(1/den)
  LamT = lam-chunks.T @ gn        (512,512)
  zT = sT * LamT                  (bf16)
  out[b,n] = sum_d xT.T @ Wt  +  sum_k zT.T @ Ut   (20 matmuls per PSUM tile)
"""

import numpy as np
import ml_dtypes

import concourse.mybir as mybir
import concourse.tile as tile
from concourse import bacc
from concourse.bass_utils import run_bass_kernel_spmd

B, D, K, E = 4096, 2048, 512, 8
N_CORES = 8
BS = B // N_CORES          # 512 tokens per core
P = 128
ND = D // P                # 16 d-chunks
NK = K // P                # 4 k-chunks
NN = D // 512              # 4 n-chunks of 512
NB = BS // P               # 4 b-chunks per core

BF16 = mybir.dt.bfloat16
F32 = mybir.dt.float32

_PROG = None


def _emit(tc, nc, xvd, wtd, utd, u1d, lamd, w2cd, nabd, outd):
    from contextlib import ExitStack

    with ExitStack() as ctx:
        const = ctx.enter_context(tc.tile_pool(name="const", bufs=1))
        xpool = ctx.enter_context(tc.tile_pool(name="xpool", bufs=1))
        wpool = ctx.enter_context(tc.tile_pool(name="wpool", bufs=1))
        work = ctx.enter_context(tc.tile_pool(name="work", bufs=1))
        opool = ctx.enter_context(tc.tile_pool(name="opool", bufs=2))
        ps = ctx.enter_context(tc.tile_pool(name="ps", bufs=8, space="PSUM"))

        # small constants on the GpSimd SWDGE queue (off the input stream)
        u1_sb = const.tile([P, ND], BF16, tag="u1")
        lam_sb = const.tile([E, K], BF16, tag="lam")
        nc.gpsimd.dma_start(out=lam_sb[:], in_=lamd[:])
        w2c_sb = const.tile([1, E], BF16, tag="w2c")
        nc.gpsimd.dma_start(out=w2c_sb[:], in_=w2cd[:])
        nab_sb = const.tile([1, 2], F32, tag="nab")
        nc.gpsimd.dma_start(out=nab_sb[:], in_=nabd[:])
        ones8 = const.tile([E, 1], BF16, tag="ones8")
        nc.vector.memset(ones8[:], 1.0)
        ones18 = const.tile([1, E], BF16, tag="ones18")
        nc.vector.memset(ones18[:], 1.0)
        ones18f = const.tile([1, E], F32, tag="ones18f")
        nc.vector.memset(ones18f[:], 1.0)

        # streamed inputs on the Sync HWDGE queue, in consumption order:
        # [xT|vT] combined chunks first, then W.T, then U.T.  One trigger
        # per chunk (~0.6us sequencer cost each) — transfer-bound end to end.
        xvs = []
        for dc in range(ND):
            t = xpool.tile([P, BS + K], BF16, tag=f"xv{dc}", name=f"xv{dc}")
            nc.sync.dma_start(out=t[:], in_=xvd[dc * P:(dc + 1) * P, :])
            xvs.append(t)
            if dc == 1:
                nc.sync.dma_start(out=u1_sb[:], in_=u1d[:])
        wts = []
        for dc in range(ND):
            t = wpool.tile([P, D], BF16, tag=f"wt{dc}", name=f"wt{dc}")
            nc.sync.dma_start(out=t[:], in_=wtd[dc * P:(dc + 1) * P, :])
            wts.append(t)
        uts = []
        for kc in range(NK):
            t = wpool.tile([P, D], BF16, tag=f"ut{kc}", name=f"ut{kc}")
            nc.sync.dma_start(out=t[:], in_=utd[kc * P:(kc + 1) * P, :])
            uts.append(t)

        # ---- phase 1, two narrow sweeps (3 then 2 PSUM banks): the first
        # paces with the xv DMA stream, the second runs dense off residents ----
        s1_ps = ps.tile([1, BS], F32, tag="ps", name="s1_ps")
        sps = [ps.tile([P, BS], F32, tag="ps", name=f"sp{kc}") for kc in range(NK)]
        for dc in range(ND):
            for kc in range(2):
                nc.tensor.matmul(sps[kc][:], xvs[dc][:, BS + kc * P:BS + (kc + 1) * P],
                                 xvs[dc][:, 0:BS], start=(dc == 0), stop=(dc == ND - 1))
            nc.tensor.matmul(s1_ps[:], u1_sb[:, dc:dc + 1], xvs[dc][:, 0:BS],
                             start=(dc == 0), stop=(dc == ND - 1))
        for dc in range(ND):
            for kc in range(2, NK):
                nc.tensor.matmul(sps[kc][:], xvs[dc][:, BS + kc * P:BS + (kc + 1) * P],
                                 xvs[dc][:, 0:BS], start=(dc == 0), stop=(dc == ND - 1))

        # -m[b] = min(-a*s1, -b*s1), a=max(W2), b=min(W2): exact row max shift
        s1row = work.tile([1, BS], BF16, tag="s1row")
        mneg = work.tile([1, BS], BF16, tag="mneg")
        ta = work.tile([1, BS], F32, tag="ta")
        tb = work.tile([1, BS], F32, tag="tb")
        nc.vector.tensor_copy(s1row[:], s1_ps[:])
        nc.vector.tensor_scalar_mul(ta[:], s1_ps[:], nab_sb[:, 0:1])
        nc.vector.tensor_scalar_mul(tb[:], s1_ps[:], nab_sb[:, 1:2])
        nc.vector.tensor_tensor(mneg[:], ta[:], tb[:], mybir.AluOpType.min)
        s_sb = []
        for kc in range(NK):
            t = work.tile([P, BS], F32, tag=f"s{kc}", name=f"s{kc}")
            nc.vector.tensor_copy(t[:], sps[kc][:])
            s_sb.append(t)

        # SBUF staging for the gating chain (filled while bc0 W-matmuls run)
        g_sb = work.tile([E, BS], BF16, tag="g")
        rden = work.tile([1, BS], F32, tag="rden")
        gn_sb = work.tile([E, BS], BF16, tag="gn")

        def emit_lam_z(kc, pstate):
            lp = ps.tile([P, BS], F32, tag="ps", name=f"lp{kc}")
            nc.tensor.matmul(lp[:], lam_sb[:, kc * P:(kc + 1) * P],
                             gn_sb[:], start=True, stop=True)
            zt = work.tile([P, BS], BF16, tag=f"z{kc}", name=f"z{kc}")
            nc.vector.tensor_tensor(zt[:], s_sb[kc][:], lp[:],
                                    mybir.AluOpType.mult)
            pstate["z"].append(zt)

        def emit_gate_mm(step, pstate):
            # tiny router matmuls spread through bc0's W-loop; their ACT/DVE
            # producers run in the shadow of the surrounding big matmuls
            if step == 0:
                e_ps = ps.tile([E, BS], F32, tag="ps", name="e_ps")
                nc.tensor.matmul(e_ps[:], w2c_sb[:], s1row[:], start=True, stop=False)
                nc.tensor.matmul(e_ps[:], ones18[:], mneg[:], start=False, stop=True)
                pstate["e_ps"] = e_ps
            elif step == 1:
                nc.scalar.activation(g_sb[:], pstate["e_ps"][:],
                                     mybir.ActivationFunctionType.Exp)
            elif step == 2:
                den_ps = ps.tile([1, BS], F32, tag="ps", name="den_ps")
                nc.tensor.matmul(den_ps[:], ones8[:], g_sb[:], start=True, stop=True)
                pstate["den_ps"] = den_ps
            elif step == 3:
                rden_f = work.tile([1, BS], F32, tag="rden_f")
                nc.vector.tensor_copy(rden_f[:], pstate["den_ps"][:])
                nc.vector.reciprocal_approx_fast(out=rden[:], in_=rden_f[:])
            elif step == 4:
                r8_ps = ps.tile([E, BS], F32, tag="ps", name="r8_ps")
                nc.tensor.matmul(r8_ps[:], ones18f[:], rden[:], start=True, stop=True)
                pstate["r8_ps"] = r8_ps
            elif step == 5:
                nc.vector.tensor_tensor(gn_sb[:], g_sb[:], pstate["r8_ps"][:],
                                        mybir.AluOpType.mult)

        # ---- main pass: out = x @ W'.T + z @ U'.T, bc0 first with gating
        # spread through it ----
        pstate = {"z": []}
        gate_at = {1: 0, 3: 1, 5: 2, 7: 3, 9: 4, 11: 5}
        lam_at = {12: 0, 13: 1, 14: 2, 15: 3}
        all_psums = []

        def emit_w_block(bc):
            psums = [ps.tile([P, 512], F32, tag="ps", name=f"po{bc}_{i}")
                     for i in range(NN)]
            all_psums.append(psums)
            for dc in range(ND):
                lhs = xvs[dc][:, bc * P:(bc + 1) * P]
                for ni in range(NN):
                    nc.tensor.matmul(psums[ni][:], lhs,
                                     wts[dc][:, ni * 512:(ni + 1) * 512],
                                     start=(dc == 0), stop=False)
                if bc == 0 and dc in gate_at:
                    emit_gate_mm(gate_at[dc], pstate)
                if bc == 0 and dc in lam_at:
                    emit_lam_z(lam_at[dc], pstate)

        def emit_u_block(bc):
            z_sb = pstate["z"]
            psums = all_psums[bc]
            o_sb = opool.tile([P, D], F32, tag="o", name=f"o{bc}")
            for ni in range(NN):
                for kc in range(NK):
                    nc.tensor.matmul(psums[ni][:],
                                     z_sb[kc][:, bc * P:(bc + 1) * P],
                                     uts[kc][:, ni * 512:(ni + 1) * 512],
                                     start=False, stop=(kc == NK - 1))
                nc.vector.tensor_copy(o_sb[:, ni * 512:(ni + 1) * 512], psums[ni][:])
                nc.scalar.dma_start(
                    out=outd[bc * P:(bc + 1) * P, ni * 512:(ni + 1) * 512],
                    in_=o_sb[:, ni * 512:(ni + 1) * 512])

        # U lags one W block so the ut/z dependencies are off the critical
        # path; at most two bc PSUM groups (8 banks) are ever live
        emit_w_block(0)
        emit_w_block(1)
        emit_u_block(0)
        emit_w_block(2)
        emit_u_block(1)
        emit_w_block(3)
        emit_u_block(2)
        emit_u_block(3)


def build_program():
    nc = bacc.Bacc("TRN2", target_bir_lowering=False, debug=False)
    xvd = nc.dram_tensor("xv", (D, BS + K), BF16, kind="ExternalInput").ap()
    wtd = nc.dram_tensor("wt", (D, D), BF16, kind="ExternalInput").ap()
    utd = nc.dram_tensor("ut", (K, D), BF16, kind="ExternalInput").ap()
    u1d = nc.dram_tensor("u1", (P, ND), BF16, kind="ExternalInput").ap()
    lamd = nc.dram_tensor("lam", (E, K), BF16, kind="ExternalInput").ap()
    w2cd = nc.dram_tensor("w2c", (1, E), BF16, kind="ExternalInput").ap()
    nabd = nc.dram_tensor("nab", (1, 2), F32, kind="ExternalInput").ap()
    outd = nc.dram_tensor("out", (BS, D), F32, kind="ExternalOutput").ap()

    with tile.TileContext(nc) as tc:
        _emit(tc, nc, xvd, wtd, utd, u1d, lamd, w2cd, nabd, outd)
    nc.compile()
    return nc


def _get_prog():
    global _PROG
    if _PROG is None:
        _PROG = build_program()
    return _PROG


def make_in_maps(x, W, U_k, V_k, lambda_k, v, router_W1, router_W2):
    bf = ml_dtypes.bfloat16
    x = np.asarray(x, dtype=np.float32)
    W = np.asarray(W, dtype=np.float32)
    U_k = np.asarray(U_k, dtype=np.float32)
    V_k = np.asarray(V_k, dtype=np.float32)
    lambda_k = np.asarray(lambda_k, dtype=np.float32)
    v = np.asarray(v, dtype=np.float32)
    router_W1 = np.asarray(router_W1, dtype=np.float32)
    router_W2 = np.asarray(router_W2, dtype=np.float32)

    scale = 1.0 + v                                       # (D,) per output row n
    wt = np.ascontiguousarray((W * scale[:, None]).T).astype(bf)     # (d, n)
    ut = np.ascontiguousarray((U_k * scale[:, None]).T).astype(bf)   # (k, n)
    vt = V_k.T.astype(bf)                                            # (d, k)
    u1 = (U_k.astype(np.float64) @ router_W1.astype(np.float64)).astype(np.float32)
    u1 = np.ascontiguousarray(u1.reshape(ND, P).T).astype(bf)        # (P, ND)
    lam = np.ascontiguousarray(lambda_k).astype(bf)                  # (E, K)
    w2 = router_W2.reshape(-1)
    w2c = np.ascontiguousarray(w2.reshape(1, E)).astype(bf)
    nab = np.array([[-w2.max(), -w2.min()]], dtype=np.float32)

    in_maps = []
    for c in range(N_CORES):
        xt = x[c * BS:(c + 1) * BS].T.astype(bf)                  # (D, BS)
        xv = np.ascontiguousarray(np.concatenate([xt, vt], axis=1))  # (D, BS+K)
        in_maps.append({"xv": xv, "wt": wt, "ut": ut, "u1": u1,
                        "lam": lam, "w2c": w2c, "nab": nab})
    return in_maps


def run(in_maps, trace=False):
    nc = _get_prog()
    res = run_bass_kernel_spmd(nc, in_maps, core_ids=list(range(N_CORES)), trace=trace)
    out = np.concatenate([res.results[c]["out"] for c in range(N_CORES)], axis=0)
    return out, res


def kernel(x, W, U_k, V_k, lambda_k, v, router_W1, router_W2):
    in_maps = make_in_maps(x, W, U_k, V_k, lambda_k, v, router_W1, router_W2)
    out, _ = run(in_maps, trace=False)
    return out



# revision 2
# speedup vs baseline: 1.0194x; 1.0194x over previous
"""MoSARA MoE-routing kernel for 8 Trainium2 NeuronCores.

Math: the reference materializes per-expert delta weights
    delta_W[e] = U_k @ diag(lambda_k[e]) @ V_k,  out = sum_e g[b,e] * x @ (W+delta_W[e]).T
but since softmax gates sum to 1 this collapses to
    out = (x @ W.T + ((x @ V_k.T) * (g @ lambda_k)) @ U_k.T) * (1+v)
with g = softmax_e((x @ U_k @ router_W1) * router_W2[e]).

v2 vs v1 (112us): fp8 DoubleRow for the low-rank terms.
  - phase 1 (s = x@V.T, s1 = x@u1) runs in fp8e4 DoubleRow: 256-deep
    contraction per MM, half the matmul count.  V is pre-scaled by 32
    (entries ~0.7 in fp8 range); the 1/32 is folded into lambda.
  - the correction term z @ U.T runs in fp8e5 (e5m2) DoubleRow at
    natural scale (z ~ 0.02, U ~ 0.02 are normal in e5m2), so it can
    accumulate straight into the W-term PSUM group - no combine op.
  - x/V/u1 ship as ONE interleaved fp8 stream c8[p, d2, 0:1040] =
    [x8 | 32*V | u1 | pad] so phase 1 paces on a single DMA queue.
  - inputs split across both HWDGE queues: Sync = c8 + xT(bf16),
    Scalar = wt, then output.  8 junk warm-up matmuls on memset tiles
    spin the PE HAM clock to 2.4GHz while the first DMAs land.
  - output is written bf16 (half the out-DMA), upcast on host.

Device per core (data-parallel over B, 512 tokens/core):
  warmup MMs; s1/sT via fp8-DR while c8 streams; exact-max softmax
  gating via tiny matmuls spread through bc0's W-loop; z8 = s*Lam in
  e5m2; out[b,n] = sum_d xT.T @ Wt (bf16) + sum_k z8.T @ Ut8 (e5m2-DR),
  18 matmuls per PSUM tile, U lagging W by one block.
"""

import numpy as np
import ml_dtypes

import concourse.mybir as mybir
import concourse.tile as tile
from concourse import bacc
from concourse.bass_utils import run_bass_kernel_spmd

B, D, K, E = 4096, 2048, 512, 8
N_CORES = 8
BS = B // N_CORES          # 512 tokens per core
P = 128
ND = D // P                # 16 d-chunks
NJ = ND // 2               # 8 d-pair chunks (DoubleRow)
NK = K // P                # 4 k-chunks
NN = D // 512              # 4 n-chunks of 512
NB = BS // P               # 4 b-chunks per core
C8W = BS + K + 16          # 1040 cols: [x8 | 32*V | u1pad]

BF16 = mybir.dt.bfloat16
F32 = mybir.dt.float32
F8E4 = mybir.dt.float8e4
F8E5 = mybir.dt.float8e5
DR = mybir.MatmulPerfMode.DoubleRow

_PROG = None


def _emit(tc, nc, c8d, xtd, wtd, ut8d, lamd, w2cd, nabd, outd):
    from contextlib import ExitStack

    with ExitStack() as ctx:
        const = ctx.enter_context(tc.tile_pool(name="const", bufs=1))
        xpool = ctx.enter_context(tc.tile_pool(name="xpool", bufs=1))
        wpool = ctx.enter_context(tc.tile_pool(name="wpool", bufs=1))
        work = ctx.enter_context(tc.tile_pool(name="work", bufs=1))
        opool = ctx.enter_context(tc.tile_pool(name="opool", bufs=2))
        ps = ctx.enter_context(tc.tile_pool(name="ps", bufs=8, space="PSUM"))

        # warm-up operands + small constants (memsets run in the preamble)
        wu_w = const.tile([P, P], BF16, tag="wu_w")
        nc.vector.memset(wu_w[:], 0.125)
        wu_x = const.tile([P, 512], BF16, tag="wu_x")
        nc.vector.memset(wu_x[:], 0.125)
        lam_sb = const.tile([E, K], BF16, tag="lam")
        nc.gpsimd.dma_start(out=lam_sb[:], in_=lamd[:])
        w2c_sb = const.tile([1, E], BF16, tag="w2c")
        nc.gpsimd.dma_start(out=w2c_sb[:], in_=w2cd[:])
        nab_sb = const.tile([1, 2], F32, tag="nab")
        nc.gpsimd.dma_start(out=nab_sb[:], in_=nabd[:])
        ones8 = const.tile([E, 1], BF16, tag="ones8")
        nc.vector.memset(ones8[:], 1.0)
        ones18 = const.tile([1, E], BF16, tag="ones18")
        nc.vector.memset(ones18[:], 1.0)
        ones18f = const.tile([1, E], F32, tag="ones18f")
        nc.vector.memset(ones18f[:], 1.0)

        # ---- input streams ----
        # Sync HWDGE: c8 fp8 bundle (phase 1) first, then xT bf16, then ut8.
        c8 = xpool.tile([P, ND, C8W], F8E4, tag="c8", name="c8")
        for t in range(4):
            nc.sync.dma_start(out=c8[:, 4 * t:4 * t + 4, :],
                              in_=c8d[:, 4 * t:4 * t + 4, :])
        xts = []
        for dc in range(ND):
            t = xpool.tile([P, BS], BF16, tag=f"xt{dc}", name=f"xt{dc}")
            nc.sync.dma_start(out=t[:], in_=xtd[dc * P:(dc + 1) * P, :])
            xts.append(t)
        ut8 = wpool.tile([P, NK, D], F8E5, tag="ut8", name="ut8")
        for t in range(2):
            nc.sync.dma_start(out=ut8[:, 2 * t:2 * t + 2, :],
                              in_=ut8d[:, 2 * t:2 * t + 2, :])
        # Scalar HWDGE: the wide W.T stream (its queue is free again by the
        # time output DMAs start)
        wts = []
        for dc in range(ND):
            t = wpool.tile([P, D], BF16, tag=f"wt{dc}", name=f"wt{dc}")
            nc.scalar.dma_start(out=t[:], in_=wtd[dc * P:(dc + 1) * P, :])
            wts.append(t)

        # ---- PE warm-up: junk matmuls while the first DMAs land ----
        wu_ps = ps.tile([P, 512], F32, tag="ps", name="wu_ps")
        for _ in range(8):
            nc.tensor.matmul(wu_ps[:], wu_w[:], wu_x[:], start=True, stop=True)

        # ---- phase 1: s = (32V) @ x.T and s1 = u1.T @ x.T in fp8-DR ----
        s1_ps = ps.tile([16, BS], F32, tag="ps", name="s1_ps")
        sps = [ps.tile([P, BS], F32, tag="ps", name=f"sp{kc}") for kc in range(NK)]
        for j in range(NJ):
            pair = c8[:, 2 * j:2 * j + 2, :]
            for kc in range(NK):
                nc.tensor.matmul(sps[kc][:],
                                 pair[:, :, BS + kc * P:BS + (kc + 1) * P],
                                 pair[:, :, 0:BS],
                                 start=(j == 0), stop=(j == NJ - 1), perf_mode=DR)
            nc.tensor.matmul(s1_ps[:], pair[:, :, BS + K:BS + K + 16],
                             pair[:, :, 0:BS],
                             start=(j == 0), stop=(j == NJ - 1), perf_mode=DR)

        # -m[b] = min(-a*s1, -b*s1), a=max(W2), b=min(W2): exact row max shift
        s1row = work.tile([1, BS], BF16, tag="s1row")
        mneg = work.tile([1, BS], BF16, tag="mneg")
        ta = work.tile([1, BS], F32, tag="ta")
        tb = work.tile([1, BS], F32, tag="tb")
        nc.vector.tensor_copy(s1row[:], s1_ps[0:1, :])
        nc.vector.tensor_scalar_mul(ta[:], s1_ps[0:1, :], nab_sb[:, 0:1])
        nc.vector.tensor_scalar_mul(tb[:], s1_ps[0:1, :], nab_sb[:, 1:2])
        nc.vector.tensor_tensor(mneg[:], ta[:], tb[:], mybir.AluOpType.min)
        s_sb = []
        for kc in range(NK):
            t = work.tile([P, BS], F32, tag=f"s{kc}", name=f"s{kc}")
            nc.vector.tensor_copy(t[:], sps[kc][:])
            s_sb.append(t)

        # SBUF staging for the gating chain (filled while bc0 W-matmuls run)
        g_sb = work.tile([E, BS], BF16, tag="g")
        rden = work.tile([1, BS], F32, tag="rden")
        gn_sb = work.tile([E, BS], BF16, tag="gn")
        # z8[jj][:, i, :] holds z for kc = 2*jj + i, e5m2 at natural scale
        z8 = [work.tile([P, 2, BS], F8E5, tag=f"z8{jj}", name=f"z8{jj}")
              for jj in range(2)]

        def emit_lam_z(kc, pstate):
            lp = ps.tile([P, BS], F32, tag="ps", name=f"lp{kc}")
            nc.tensor.matmul(lp[:], lam_sb[:, kc * P:(kc + 1) * P],
                             gn_sb[:], start=True, stop=True)
            nc.vector.tensor_tensor(z8[kc // 2][:, kc % 2, :], s_sb[kc][:], lp[:],
                                    mybir.AluOpType.mult)

        def emit_gate_mm(step, pstate):
            # tiny router matmuls spread through bc0's W-loop; their ACT/DVE
            # producers run in the shadow of the surrounding big matmuls
            if step == 0:
                e_ps = ps.tile([E, BS], F32, tag="ps", name="e_ps")
                nc.tensor.matmul(e_ps[:], w2c_sb[:], s1row[:], start=True, stop=False)
                nc.tensor.matmul(e_ps[:], ones18[:], mneg[:], start=False, stop=True)
                pstate["e_ps"] = e_ps
            elif step == 1:
                nc.scalar.activation(g_sb[:], pstate["e_ps"][:],
                                     mybir.ActivationFunctionType.Exp)
            elif step == 2:
                den_ps = ps.tile([1, BS], F32, tag="ps", name="den_ps")
                nc.tensor.matmul(den_ps[:], ones8[:], g_sb[:], start=True, stop=True)
                pstate["den_ps"] = den_ps
            elif step == 3:
                rden_f = work.tile([1, BS], F32, tag="rden_f")
                nc.vector.tensor_copy(rden_f[:], pstate["den_ps"][:])
                nc.vector.reciprocal_approx_fast(out=rden[:], in_=rden_f[:])
            elif step == 4:
                r8_ps = ps.tile([E, BS], F32, tag="ps", name="r8_ps")
                nc.tensor.matmul(r8_ps[:], ones18f[:], rden[:], start=True, stop=True)
                pstate["r8_ps"] = r8_ps
            elif step == 5:
                nc.vector.tensor_tensor(gn_sb[:], g_sb[:], pstate["r8_ps"][:],
                                        mybir.AluOpType.mult)

        # ---- main pass: out = x @ W'.T + z8 @ U'.T, bc0 first with gating
        # spread through it ----
        pstate = {}
        gate_at = {1: 0, 3: 1, 5: 2, 7: 3, 9: 4, 11: 5}
        lam_at = {12: 0, 13: 1, 14: 2, 15: 3}
        all_psums = []

        def emit_w_block(bc):
            psums = [ps.tile([P, 512], F32, tag="ps", name=f"po{bc}_{i}")
                     for i in range(NN)]
            all_psums.append(psums)
            for dc in range(ND):
                lhs = xts[dc][:, bc * P:(bc + 1) * P]
                for ni in range(NN):
                    nc.tensor.matmul(psums[ni][:], lhs,
                                     wts[dc][:, ni * 512:(ni + 1) * 512],
                                     start=(dc == 0), stop=False)
                if bc == 0 and dc in gate_at:
                    emit_gate_mm(gate_at[dc], pstate)
                if bc == 0 and dc in lam_at:
                    emit_lam_z(lam_at[dc], pstate)

        def emit_u_block(bc):
            psums = all_psums[bc]
            o_sb = opool.tile([P, D], BF16, tag="o", name=f"o{bc}")
            for ni in range(NN):
                for jj in range(2):
                    nc.tensor.matmul(psums[ni][:],
                                     z8[jj][:, :, bc * P:(bc + 1) * P],
                                     ut8[:, 2 * jj:2 * jj + 2,
                                         ni * 512:(ni + 1) * 512],
                                     start=False, stop=(jj == 1), perf_mode=DR)
                nc.vector.tensor_copy(o_sb[:, ni * 512:(ni + 1) * 512], psums[ni][:])
                nc.scalar.dma_start(
                    out=outd[bc * P:(bc + 1) * P, ni * 512:(ni + 1) * 512],
                    in_=o_sb[:, ni * 512:(ni + 1) * 512])

        # U lags one W block so the ut/z dependencies are off the critical
        # path; at most two bc PSUM groups (8 banks) are ever live
        emit_w_block(0)
        emit_w_block(1)
        emit_u_block(0)
        emit_w_block(2)
        emit_u_block(1)
        emit_w_block(3)
        emit_u_block(2)
        emit_u_block(3)


def build_program():
    nc = bacc.Bacc("TRN2", target_bir_lowering=False, debug=False)
    c8d = nc.dram_tensor("c8", (P, ND, C8W), F8E4, kind="ExternalInput").ap()
    xtd = nc.dram_tensor("xt", (D, BS), BF16, kind="ExternalInput").ap()
    wtd = nc.dram_tensor("wt", (D, D), BF16, kind="ExternalInput").ap()
    ut8d = nc.dram_tensor("ut8", (P, NK, D), F8E5, kind="ExternalInput").ap()
    lamd = nc.dram_tensor("lam", (E, K), BF16, kind="ExternalInput").ap()
    w2cd = nc.dram_tensor("w2c", (1, E), BF16, kind="ExternalInput").ap()
    nabd = nc.dram_tensor("nab", (1, 2), F32, kind="ExternalInput").ap()
    outd = nc.dram_tensor("out", (BS, D), BF16, kind="ExternalOutput").ap()

    with tile.TileContext(nc) as tc:
        _emit(tc, nc, c8d, xtd, wtd, ut8d, lamd, w2cd, nabd, outd)
    nc.compile()
    return nc


def _get_prog():
    global _PROG
    if _PROG is None:
        _PROG = build_program()
    return _PROG


def make_in_maps(x, W, U_k, V_k, lambda_k, v, router_W1, router_W2):
    bf = ml_dtypes.bfloat16
    f8e4 = ml_dtypes.float8_e4m3
    f8e5 = ml_dtypes.float8_e5m2
    x = np.asarray(x, dtype=np.float32)
    W = np.asarray(W, dtype=np.float32)
    U_k = np.asarray(U_k, dtype=np.float32)
    V_k = np.asarray(V_k, dtype=np.float32)
    lambda_k = np.asarray(lambda_k, dtype=np.float32)
    v = np.asarray(v, dtype=np.float32)
    router_W1 = np.asarray(router_W1, dtype=np.float32)
    router_W2 = np.asarray(router_W2, dtype=np.float32)

    scale = 1.0 + v                                       # (D,) per output row n
    wt = np.ascontiguousarray((W * scale[:, None]).T).astype(bf)     # (d, n)
    # ut8[p, kc, n] = (U*(1+v))[n, kc*128+p] in e5m2, natural scale
    ut = (U_k * scale[:, None]).T                                    # (k, n)
    ut8 = np.ascontiguousarray(
        ut.reshape(NK, P, D).transpose(1, 0, 2)).astype(f8e5)        # (P, NK, D)
    u1 = (U_k.astype(np.float64) @ router_W1.astype(np.float64)).astype(np.float32)
    lam = np.ascontiguousarray(lambda_k / 32.0).astype(bf)           # (E, K)
    w2 = router_W2.reshape(-1)
    w2c = np.ascontiguousarray(w2.reshape(1, E)).astype(bf)
    nab = np.array([[-w2.max(), -w2.min()]], dtype=np.float32)

    # c8[p, d2, :] = [ x[b, d2*128+p] | 32*V[k, d2*128+p] | u1[d2*128+p] pad ]
    v32 = (32.0 * V_k).T.reshape(ND, P, K).transpose(1, 0, 2)        # (P, ND, K)
    u1c = u1.reshape(ND, P).T[:, :, None]                            # (P, ND, 1)
    pad = np.zeros((P, ND, 15), dtype=np.float32)

    in_maps = []
    for c in range(N_CORES):
        xs = x[c * BS:(c + 1) * BS]                                  # (BS, D)
        xt = np.ascontiguousarray(xs.T).astype(bf)                   # (D, BS)
        x8 = xs.T.reshape(ND, P, BS).transpose(1, 0, 2)              # (P, ND, BS)
        c8 = np.ascontiguousarray(
            np.concatenate([x8, v32, u1c, pad], axis=2)).astype(f8e4)
        in_maps.append({"c8": c8, "xt": xt, "wt": wt, "ut8": ut8,
                        "lam": lam, "w2c": w2c, "nab": nab})
    return in_maps


def run(in_maps, trace=False):
    nc = _get_prog()
    res = run_bass_kernel_spmd(nc, in_maps, core_ids=list(range(N_CORES)), trace=trace)
    out = np.concatenate(
        [res.results[c]["out"].astype(np.float32) for c in range(N_CORES)], axis=0)
    return out, res


def kernel(x, W, U_k, V_k, lambda_k, v, router_W1, router_W2):
    in_maps = make_in_maps(x, W, U_k, V_k, lambda_k, v, router_W1, router_W2)
    out, _ = run(in_maps, trace=False)
    return out


# revision 8
# speedup vs baseline: 1.1258x; 1.1043x over previous
"""MoSARA MoE-routing kernel for 8 Trainium2 NeuronCores.

Math: the reference materializes per-expert delta weights
    delta_W[e] = U_k @ diag(lambda_k[e]) @ V_k,  out = sum_e g[b,e] * x @ (W+delta_W[e]).T
but since softmax gates sum to 1 this collapses to
    out = (x @ W.T + ((x @ V_k.T) * (g @ lambda_k)) @ U_k.T) * (1+v)
with g = softmax_e((x @ U_k @ router_W1) * router_W2[e]).

v2 vs v1 (112us): fp8 DoubleRow for the low-rank terms.
  - phase 1 (s = x@V.T, s1 = x@u1) runs in fp8e4 DoubleRow: 256-deep
    contraction per MM, half the matmul count.  V is pre-scaled by 32
    (entries ~0.7 in fp8 range); the 1/32 is folded into lambda.
  - the correction term z @ U.T runs in fp8e5 (e5m2) DoubleRow at
    natural scale (z ~ 0.02, U ~ 0.02 are normal in e5m2), so it can
    accumulate straight into the W-term PSUM group - no combine op.
  - x/V/u1 ship as ONE interleaved fp8 stream c8[p, d2, 0:1040] =
    [x8 | 32*V | u1 | pad] so phase 1 paces on a single DMA queue.
  - inputs split across both HWDGE queues: Sync = c8 + xT(bf16),
    Scalar = wt, then output.  8 junk warm-up matmuls on memset tiles
    spin the PE HAM clock to 2.4GHz while the first DMAs land.
  - output is written bf16 (half the out-DMA), upcast on host.

Device per core (data-parallel over B, 512 tokens/core):
  warmup MMs; s1/sT via fp8-DR while c8 streams; exact-max softmax
  gating via tiny matmuls spread through bc0's W-loop; z8 = s*Lam in
  e5m2; out[b,n] = sum_d xT.T @ Wt (bf16) + sum_k z8.T @ Ut8 (e5m2-DR),
  18 matmuls per PSUM tile, U lagging W by one block.
"""

import numpy as np
import ml_dtypes

import concourse.mybir as mybir
import concourse.tile as tile
from concourse import bacc
from concourse.bass_utils import run_bass_kernel_spmd

B, D, K, E = 4096, 2048, 512, 8
N_CORES = 8
BS = B // N_CORES          # 512 tokens per core
P = 128
ND = D // P                # 16 d-chunks
NJ = ND // 2               # 8 d-pair chunks (DoubleRow)
NK = K // P                # 4 k-chunks
NN = D // 512              # 4 n-chunks of 512
NB = BS // P               # 4 b-chunks per core
C8W = BS + K + 16          # 1040 cols: [x8 | 32*V | u1pad]

BF16 = mybir.dt.bfloat16
F32 = mybir.dt.float32
F8E4 = mybir.dt.float8e4
F8E5 = mybir.dt.float8e5
DR = mybir.MatmulPerfMode.DoubleRow

_PROG = None


def _emit(tc, nc, c8d, xwd, ut8d, lamd, w2cd, nabd, outd):
    from contextlib import ExitStack

    with ExitStack() as ctx:
        const = ctx.enter_context(tc.tile_pool(name="const", bufs=1))
        xpool = ctx.enter_context(tc.tile_pool(name="xpool", bufs=1))
        wpool = ctx.enter_context(tc.tile_pool(name="wpool", bufs=1))
        work = ctx.enter_context(tc.tile_pool(name="work", bufs=1))
        opool = ctx.enter_context(tc.tile_pool(name="opool", bufs=2))
        ps = ctx.enter_context(tc.tile_pool(name="ps", bufs=8, space="PSUM"))

        # warm-up operands + small constants (memsets run in the preamble)
        wu_w = const.tile([P, P], BF16, tag="wu_w")
        nc.vector.memset(wu_w[:], 0.125)
        wu_x = const.tile([P, 512], BF16, tag="wu_x")
        nc.vector.memset(wu_x[:], 0.125)
        lam_sb = const.tile([E, K], BF16, tag="lam")
        nc.gpsimd.dma_start(out=lam_sb[:], in_=lamd[:])
        w2c_sb = const.tile([1, E], BF16, tag="w2c")
        nc.gpsimd.dma_start(out=w2c_sb[:], in_=w2cd[:])
        nab_sb = const.tile([1, 2], F32, tag="nab")
        nc.gpsimd.dma_start(out=nab_sb[:], in_=nabd[:])
        ones8 = const.tile([E, 1], BF16, tag="ones8")
        nc.vector.memset(ones8[:], 1.0)
        ones18 = const.tile([1, E], BF16, tag="ones18")
        nc.vector.memset(ones18[:], 1.0)
        ones18f = const.tile([1, E], F32, tag="ones18f")
        nc.vector.memset(ones18f[:], 1.0)

        # ---- input stream: ONE Sync HWDGE queue in exact consumption order.
        # The aggregate HBM read (~12.6MB) is bandwidth-bound; any queue
        # split or misordering starves the PE (v2 lost ~20us to wt-starved
        # W-blocks + HAM re-throttle).  [x|W] merge keeps per-dc arrival
        # granularity with half the DMA triggers.
        c8 = xpool.tile([P, ND, C8W], F8E4, tag="c8", name="c8")
        xws = [xpool.tile([P, BS + D], BF16, tag=f"xw{dc}", name=f"xw{dc}")
               for dc in range(ND)]

        def dma_c8(t):
            nc.sync.dma_start(out=c8[:, 4 * t:4 * t + 4, :],
                              in_=c8d[:, 4 * t:4 * t + 4, :])

        def dma_xw(dc):
            nc.sync.dma_start(out=xws[dc][:], in_=xwd[dc * P:(dc + 1) * P, :])

        dma_c8(0)
        dma_xw(0)
        dma_c8(1)
        dma_xw(1)
        dma_c8(2)
        dma_xw(2)
        dma_c8(3)
        for dc in range(3, ND):
            dma_xw(dc)
        ut8 = wpool.tile([P, NK, D], F8E5, tag="ut8", name="ut8")
        for t in range(2):
            nc.sync.dma_start(out=ut8[:, 2 * t:2 * t + 2, :],
                              in_=ut8d[:, 2 * t:2 * t + 2, :])

        # ---- PE warm-up: junk matmuls while the first DMAs land ----
        wu_ps = ps.tile([P, 512], F32, tag="ps", name="wu_ps")
        for _ in range(8):
            nc.tensor.matmul(wu_ps[:], wu_w[:], wu_x[:], start=True, stop=True)

        # ---- phase 1: s = (32V) @ x.T and s1 = u1.T @ x.T in fp8-DR ----
        s1_ps = ps.tile([16, BS], F32, tag="ps", name="s1_ps")
        sps = [ps.tile([P, BS], F32, tag="ps", name=f"sp{kc}") for kc in range(NK)]
        for j in range(NJ):
            pair = c8[:, 2 * j:2 * j + 2, :]
            for kc in range(NK):
                nc.tensor.matmul(sps[kc][:],
                                 pair[:, :, BS + kc * P:BS + (kc + 1) * P],
                                 pair[:, :, 0:BS],
                                 start=(j == 0), stop=(j == NJ - 1), perf_mode=DR)
            nc.tensor.matmul(s1_ps[:], pair[:, :, BS + K:BS + K + 16],
                             pair[:, :, 0:BS],
                             start=(j == 0), stop=(j == NJ - 1), perf_mode=DR)

        # -m[b] = min(-a*s1, -b*s1), a=max(W2), b=min(W2): exact row max shift
        s1row = work.tile([1, BS], BF16, tag="s1row")
        mneg = work.tile([1, BS], BF16, tag="mneg")
        ta = work.tile([1, BS], F32, tag="ta")
        tb = work.tile([1, BS], F32, tag="tb")
        nc.vector.tensor_copy(s1row[:], s1_ps[0:1, :])
        nc.vector.tensor_scalar_mul(ta[:], s1_ps[0:1, :], nab_sb[:, 0:1])
        nc.vector.tensor_scalar_mul(tb[:], s1_ps[0:1, :], nab_sb[:, 1:2])
        nc.vector.tensor_tensor(mneg[:], ta[:], tb[:], mybir.AluOpType.min)
        s_sb = []
        for kc in range(NK):
            t = work.tile([P, BS], F32, tag=f"s{kc}", name=f"s{kc}")
            nc.vector.tensor_copy(t[:], sps[kc][:])
            s_sb.append(t)

        # SBUF staging for the gating chain (filled while bc0 W-matmuls run)
        g_sb = work.tile([E, BS], BF16, tag="g")
        rden = work.tile([1, BS], F32, tag="rden")
        gn_sb = work.tile([E, BS], BF16, tag="gn")
        # z8[jj][:, i, :] holds z for kc = 2*jj + i, e5m2 at natural scale
        z8 = [work.tile([P, 2, BS], F8E5, tag=f"z8{jj}", name=f"z8{jj}")
              for jj in range(2)]

        def emit_lam_z(kc, pstate):
            lp = ps.tile([P, BS], F32, tag="ps", name=f"lp{kc}")
            nc.tensor.matmul(lp[:], lam_sb[:, kc * P:(kc + 1) * P],
                             gn_sb[:], start=True, stop=True)
            nc.vector.tensor_tensor(z8[kc // 2][:, kc % 2, :], s_sb[kc][:], lp[:],
                                    mybir.AluOpType.mult)

        def emit_gate_mm(step, pstate):
            # tiny router matmuls spread through bc0's W-loop; their ACT/DVE
            # producers run in the shadow of the surrounding big matmuls
            if step == 0:
                e_ps = ps.tile([E, BS], F32, tag="ps", name="e_ps")
                nc.tensor.matmul(e_ps[:], w2c_sb[:], s1row[:], start=True, stop=False)
                nc.tensor.matmul(e_ps[:], ones18[:], mneg[:], start=False, stop=True)
                pstate["e_ps"] = e_ps
            elif step == 1:
                nc.scalar.activation(g_sb[:], pstate["e_ps"][:],
                                     mybir.ActivationFunctionType.Exp)
            elif step == 2:
                den_ps = ps.tile([1, BS], F32, tag="ps", name="den_ps")
                nc.tensor.matmul(den_ps[:], ones8[:], g_sb[:], start=True, stop=True)
                pstate["den_ps"] = den_ps
            elif step == 3:
                rden_f = work.tile([1, BS], F32, tag="rden_f")
                nc.vector.tensor_copy(rden_f[:], pstate["den_ps"][:])
                nc.vector.reciprocal_approx_fast(out=rden[:], in_=rden_f[:])
            elif step == 4:
                r8_ps = ps.tile([E, BS], F32, tag="ps", name="r8_ps")
                nc.tensor.matmul(r8_ps[:], ones18f[:], rden[:], start=True, stop=True)
                pstate["r8_ps"] = r8_ps
            elif step == 5:
                nc.vector.tensor_tensor(gn_sb[:], g_sb[:], pstate["r8_ps"][:],
                                        mybir.AluOpType.mult)

        # ---- main pass: out = x @ W'.T + z8 @ U'.T, bc0 first with gating
        # spread through it ----
        pstate = {}
        gate_at = {1: 0, 3: 1, 5: 2, 7: 3, 9: 4, 11: 5}
        lam_at = {12: 0, 13: 1, 14: 2, 15: 3}
        all_psums = []

        def emit_w_block(bc):
            psums = [ps.tile([P, 512], F32, tag="ps", name=f"po{bc}_{i}")
                     for i in range(NN)]
            all_psums.append(psums)
            for dc in range(ND):
                lhs = xws[dc][:, bc * P:(bc + 1) * P]
                for ni in range(NN):
                    nc.tensor.matmul(psums[ni][:], lhs,
                                     xws[dc][:, BS + ni * 512:BS + (ni + 1) * 512],
                                     start=(dc == 0), stop=False)
                if bc == 0 and dc in gate_at:
                    emit_gate_mm(gate_at[dc], pstate)
                if bc == 0 and dc in lam_at:
                    emit_lam_z(lam_at[dc], pstate)

        def emit_u_block(bc):
            psums = all_psums[bc]
            o_sb = opool.tile([P, D], BF16, tag="o", name=f"o{bc}")
            for ni in range(NN):
                for jj in range(2):
                    nc.tensor.matmul(psums[ni][:],
                                     z8[jj][:, :, bc * P:(bc + 1) * P],
                                     ut8[:, 2 * jj:2 * jj + 2,
                                         ni * 512:(ni + 1) * 512],
                                     start=False, stop=(jj == 1), perf_mode=DR)
                nc.vector.tensor_copy(o_sb[:, ni * 512:(ni + 1) * 512], psums[ni][:])
                nc.scalar.dma_start(
                    out=outd[bc * P:(bc + 1) * P, ni * 512:(ni + 1) * 512],
                    in_=o_sb[:, ni * 512:(ni + 1) * 512])

        # U lags one W block so the ut/z dependencies are off the critical
        # path; at most two bc PSUM groups (8 banks) are ever live
        emit_w_block(0)
        emit_w_block(1)
        emit_u_block(0)
        emit_w_block(2)
        emit_u_block(1)
        emit_w_block(3)
        emit_u_block(2)
        emit_u_block(3)


def build_program():
    nc = bacc.Bacc("TRN2", target_bir_lowering=False, debug=False)
    c8d = nc.dram_tensor("c8", (P, ND, C8W), F8E4, kind="ExternalInput").ap()
    xwd = nc.dram_tensor("xw", (D, BS + D), BF16, kind="ExternalInput").ap()
    ut8d = nc.dram_tensor("ut8", (P, NK, D), F8E5, kind="ExternalInput").ap()
    lamd = nc.dram_tensor("lam", (E, K), BF16, kind="ExternalInput").ap()
    w2cd = nc.dram_tensor("w2c", (1, E), BF16, kind="ExternalInput").ap()
    nabd = nc.dram_tensor("nab", (1, 2), F32, kind="ExternalInput").ap()
    outd = nc.dram_tensor("out", (BS, D), BF16, kind="ExternalOutput").ap()

    with tile.TileContext(nc) as tc:
        _emit(tc, nc, c8d, xwd, ut8d, lamd, w2cd, nabd, outd)
    nc.compile()
    return nc


def _get_prog():
    global _PROG
    if _PROG is None:
        _PROG = build_program()
    return _PROG


def make_in_maps(x, W, U_k, V_k, lambda_k, v, router_W1, router_W2):
    bf = ml_dtypes.bfloat16
    f8e4 = ml_dtypes.float8_e4m3
    f8e5 = ml_dtypes.float8_e5m2
    x = np.asarray(x, dtype=np.float32)
    W = np.asarray(W, dtype=np.float32)
    U_k = np.asarray(U_k, dtype=np.float32)
    V_k = np.asarray(V_k, dtype=np.float32)
    lambda_k = np.asarray(lambda_k, dtype=np.float32)
    v = np.asarray(v, dtype=np.float32)
    router_W1 = np.asarray(router_W1, dtype=np.float32)
    router_W2 = np.asarray(router_W2, dtype=np.float32)

    scale = 1.0 + v                                       # (D,) per output row n
    wt = np.ascontiguousarray((W * scale[:, None]).T).astype(bf)     # (d, n)
    # ut8[p, kc, n] = (U*(1+v))[n, kc*128+p] in e5m2, natural scale
    ut = (U_k * scale[:, None]).T                                    # (k, n)
    ut8 = np.ascontiguousarray(
        ut.reshape(NK, P, D).transpose(1, 0, 2)).astype(f8e5)        # (P, NK, D)
    u1 = (U_k.astype(np.float64) @ router_W1.astype(np.float64)).astype(np.float32)
    lam = np.ascontiguousarray(lambda_k / 32.0).astype(bf)           # (E, K)
    w2 = router_W2.reshape(-1)
    w2c = np.ascontiguousarray(w2.reshape(1, E)).astype(bf)
    nab = np.array([[-w2.max(), -w2.min()]], dtype=np.float32)

    # c8[p, d2, :] = [ x[b, d2*128+p] | 32*V[k, d2*128+p] | u1[d2*128+p] pad ]
    v32 = (32.0 * V_k).T.reshape(ND, P, K).transpose(1, 0, 2)        # (P, ND, K)
    u1c = u1.reshape(ND, P).T[:, :, None]                            # (P, ND, 1)
    pad = np.zeros((P, ND, 15), dtype=np.float32)

    in_maps = []
    for c in range(N_CORES):
        xs = x[c * BS:(c + 1) * BS]                                  # (BS, D)
        xw = np.ascontiguousarray(
            np.concatenate([xs.T.astype(bf), wt], axis=1))           # (D, BS+D)
        x8 = xs.T.reshape(ND, P, BS).transpose(1, 0, 2)              # (P, ND, BS)
        c8 = np.ascontiguousarray(
            np.concatenate([x8, v32, u1c, pad], axis=2)).astype(f8e4)
        in_maps.append({"c8": c8, "xw": xw, "ut8": ut8,
                        "lam": lam, "w2c": w2c, "nab": nab})
    return in_maps


def run(in_maps, trace=False):
    nc = _get_prog()
    res = run_bass_kernel_spmd(nc, in_maps, core_ids=list(range(N_CORES)), trace=trace)
    out = np.concatenate(
        [res.results[c]["out"].astype(np.float32) for c in range(N_CORES)], axis=0)
    return out, res


def kernel(x, W, U_k, V_k, lambda_k, v, router_W1, router_W2):
    in_maps = make_in_maps(x, W, U_k, V_k, lambda_k, v, router_W1, router_W2)
    out, _ = run(in_maps, trace=False)
    return out


# revision 12
# speedup vs baseline: 1.1609x; 1.0312x over previous
"""MoSARA MoE-routing kernel for 8 Trainium2 NeuronCores.

Math: the reference materializes per-expert delta weights
    delta_W[e] = U_k @ diag(lambda_k[e]) @ V_k,  out = sum_e g[b,e] * x @ (W+delta_W[e]).T
but since softmax gates sum to 1 this collapses to
    out = (x @ W.T + ((x @ V_k.T) * (g @ lambda_k)) @ U_k.T) * (1+v)
with g = softmax_e((x @ U_k @ router_W1) * router_W2[e]).

v2 vs v1 (112us): fp8 DoubleRow for the low-rank terms.
  - phase 1 (s = x@V.T, s1 = x@u1) runs in fp8e4 DoubleRow: 256-deep
    contraction per MM, half the matmul count.  V is pre-scaled by 32
    (entries ~0.7 in fp8 range); the 1/32 is folded into lambda.
  - the correction term z @ U.T runs in fp8e5 (e5m2) DoubleRow at
    natural scale (z ~ 0.02, U ~ 0.02 are normal in e5m2), so it can
    accumulate straight into the W-term PSUM group - no combine op.
  - x/V/u1 ship as ONE interleaved fp8 stream c8[p, d2, 0:1040] =
    [x8 | 32*V | u1 | pad] so phase 1 paces on a single DMA queue.
  - inputs split across both HWDGE queues: Sync = c8 + xT(bf16),
    Scalar = wt, then output.  8 junk warm-up matmuls on memset tiles
    spin the PE HAM clock to 2.4GHz while the first DMAs land.
  - output is written bf16 (half the out-DMA), upcast on host.

Device per core (data-parallel over B, 512 tokens/core):
  warmup MMs; s1/sT via fp8-DR while c8 streams; exact-max softmax
  gating via tiny matmuls spread through bc0's W-loop; z8 = s*Lam in
  e5m2; out[b,n] = sum_d xT.T @ Wt (bf16) + sum_k z8.T @ Ut8 (e5m2-DR),
  18 matmuls per PSUM tile, U lagging W by one block.
"""

import numpy as np
import ml_dtypes

import concourse.mybir as mybir
import concourse.tile as tile
from concourse import bacc
from concourse.bass_utils import run_bass_kernel_spmd

B, D, K, E = 4096, 2048, 512, 8
N_CORES = 8
BS = B // N_CORES          # 512 tokens per core
P = 128
ND = D // P                # 16 d-chunks
NJ = ND // 2               # 8 d-pair chunks (DoubleRow)
NK = K // P                # 4 k-chunks
NN = D // 512              # 4 n-chunks of 512
NB = BS // P               # 4 b-chunks per core
C8W = BS + K + 16          # 1040 cols: [x8 | 32*V | u1pad]

BF16 = mybir.dt.bfloat16
F32 = mybir.dt.float32
F8E4 = mybir.dt.float8e4
F8E5 = mybir.dt.float8e5
DR = mybir.MatmulPerfMode.DoubleRow

_PROG = None


def _emit(tc, nc, c8d, xwd, ut8d, lamd, w2cd, nabd, outd):
    from contextlib import ExitStack

    with ExitStack() as ctx:
        const = ctx.enter_context(tc.tile_pool(name="const", bufs=1))
        xpool = ctx.enter_context(tc.tile_pool(name="xpool", bufs=1))
        wpool = ctx.enter_context(tc.tile_pool(name="wpool", bufs=1))
        work = ctx.enter_context(tc.tile_pool(name="work", bufs=1))
        opool = ctx.enter_context(tc.tile_pool(name="opool", bufs=2))
        ps = ctx.enter_context(tc.tile_pool(name="ps", bufs=8, space="PSUM"))

        # warm-up operands + small constants (memsets split across engines so
        # both land right after the preamble barrier)
        wu_w = const.tile([P, P], BF16, tag="wu_w")
        nc.vector.memset(wu_w[:], 0.125)
        wu_x = const.tile([P, 512], BF16, tag="wu_x")
        nc.gpsimd.memset(wu_x[:], 0.125)
        lam_sb = const.tile([E, K], BF16, tag="lam")
        nc.gpsimd.dma_start(out=lam_sb[:], in_=lamd[:])
        w2c_sb = const.tile([1, E], BF16, tag="w2c")
        nc.gpsimd.dma_start(out=w2c_sb[:], in_=w2cd[:])
        nab_sb = const.tile([1, 2], F32, tag="nab")
        nc.gpsimd.dma_start(out=nab_sb[:], in_=nabd[:])
        ones8 = const.tile([E, 1], BF16, tag="ones8")
        nc.vector.memset(ones8[:], 1.0)
        ones18 = const.tile([1, E], BF16, tag="ones18")
        nc.vector.memset(ones18[:], 1.0)
        ones18f = const.tile([1, E], F32, tag="ones18f")
        nc.vector.memset(ones18f[:], 1.0)

        # ---- input stream: ONE Sync HWDGE queue in exact consumption order.
        # The aggregate HBM read (~12.6MB) is bandwidth-bound; any queue
        # split or misordering starves the PE (v2 lost ~20us to wt-starved
        # W-blocks + HAM re-throttle).  [x|W] merge keeps per-dc arrival
        # granularity with half the DMA triggers.
        c8 = xpool.tile([P, ND, C8W], F8E4, tag="c8", name="c8")
        xws = [xpool.tile([P, BS + D], BF16, tag=f"xw{dc}", name=f"xw{dc}")
               for dc in range(ND)]

        def dma_c8(lo, hi):
            nc.sync.dma_start(out=c8[:, lo:hi, :], in_=c8d[:, lo:hi, :])

        def dma_xw(dc):
            nc.sync.dma_start(out=xws[dc][:], in_=xwd[dc * P:(dc + 1) * P, :])

        # small head chunk so phase 1 starts as early as the DMA latency
        # allows; all of c8 lands before the first xw chunks are needed
        dma_c8(0, 2)
        dma_c8(2, 4)
        dma_c8(4, 8)
        dma_c8(8, 12)
        dma_c8(12, 16)
        for dc in range(ND):
            dma_xw(dc)
        ut8 = wpool.tile([P, NK, D], F8E5, tag="ut8", name="ut8")
        for t in range(2):
            nc.sync.dma_start(out=ut8[:, 2 * t:2 * t + 2, :],
                              in_=ut8d[:, 2 * t:2 * t + 2, :])

        # ---- PE warm-up: junk matmuls while the first DMAs land (the HAM
        # clock-gate needs ~3.4us of PE activity to reach 2.4GHz, and the
        # first c8 chunk takes ~4.5us to arrive) ----
        wu_ps = ps.tile([P, 512], F32, tag="ps", name="wu_ps")
        for _ in range(10):
            nc.tensor.matmul(wu_ps[:], wu_w[:], wu_x[:], start=True, stop=True)

        # ---- phase 1: s = (32V) @ x.T and s1 = u1.T @ x.T in fp8-DR ----
        s1_ps = ps.tile([16, BS], F32, tag="ps", name="s1_ps")
        sps = [ps.tile([P, BS], F32, tag="ps", name=f"sp{kc}") for kc in range(NK)]
        for j in range(NJ):
            pair = c8[:, 2 * j:2 * j + 2, :]
            for kc in range(NK):
                nc.tensor.matmul(sps[kc][:],
                                 pair[:, :, BS + kc * P:BS + (kc + 1) * P],
                                 pair[:, :, 0:BS],
                                 start=(j == 0), stop=(j == NJ - 1), perf_mode=DR)
            nc.tensor.matmul(s1_ps[:], pair[:, :, BS + K:BS + K + 16],
                             pair[:, :, 0:BS],
                             start=(j == 0), stop=(j == NJ - 1), perf_mode=DR)

        # -m[b] = min(-a*s1, -b*s1), a=max(W2), b=min(W2): exact row max shift
        s1row = work.tile([1, BS], BF16, tag="s1row")
        mneg = work.tile([1, BS], BF16, tag="mneg")
        ta = work.tile([1, BS], F32, tag="ta")
        tb = work.tile([1, BS], F32, tag="tb")
        nc.vector.tensor_copy(s1row[:], s1_ps[0:1, :])
        nc.vector.tensor_scalar_mul(ta[:], s1_ps[0:1, :], nab_sb[:, 0:1])
        nc.vector.tensor_scalar_mul(tb[:], s1_ps[0:1, :], nab_sb[:, 1:2])
        nc.vector.tensor_tensor(mneg[:], ta[:], tb[:], mybir.AluOpType.min)
        s_sb = []
        for kc in range(NK):
            t = work.tile([P, BS], F32, tag=f"s{kc}", name=f"s{kc}")
            nc.vector.tensor_copy(t[:], sps[kc][:])
            s_sb.append(t)

        # SBUF staging for the gating chain (filled while bc0 W-matmuls run)
        g_sb = work.tile([E, BS], BF16, tag="g")
        rden = work.tile([1, BS], F32, tag="rden")
        gn_sb = work.tile([E, BS], BF16, tag="gn")
        # z8[jj][:, i, :] holds z for kc = 2*jj + i, e5m2 at natural scale
        z8 = [work.tile([P, 2, BS], F8E5, tag=f"z8{jj}", name=f"z8{jj}")
              for jj in range(2)]

        def emit_lam_z(kc, pstate):
            lp = ps.tile([P, BS], F32, tag="ps", name=f"lp{kc}")
            nc.tensor.matmul(lp[:], lam_sb[:, kc * P:(kc + 1) * P],
                             gn_sb[:], start=True, stop=True)
            nc.vector.tensor_tensor(z8[kc // 2][:, kc % 2, :], s_sb[kc][:], lp[:],
                                    mybir.AluOpType.mult)

        def emit_gate_mm(step, pstate):
            # tiny router matmuls spread through bc0's W-loop; their ACT/DVE
            # producers run in the shadow of the surrounding big matmuls
            if step == 0:
                e_ps = ps.tile([E, BS], F32, tag="ps", name="e_ps")
                nc.tensor.matmul(e_ps[:], w2c_sb[:], s1row[:], start=True, stop=False)
                nc.tensor.matmul(e_ps[:], ones18[:], mneg[:], start=False, stop=True)
                pstate["e_ps"] = e_ps
            elif step == 1:
                nc.scalar.activation(g_sb[:], pstate["e_ps"][:],
                                     mybir.ActivationFunctionType.Exp)
            elif step == 2:
                den_ps = ps.tile([1, BS], F32, tag="ps", name="den_ps")
                nc.tensor.matmul(den_ps[:], ones8[:], g_sb[:], start=True, stop=True)
                pstate["den_ps"] = den_ps
            elif step == 3:
                rden_f = work.tile([1, BS], F32, tag="rden_f")
                nc.vector.tensor_copy(rden_f[:], pstate["den_ps"][:])
                nc.vector.reciprocal_approx_fast(out=rden[:], in_=rden_f[:])
            elif step == 4:
                r8_ps = ps.tile([E, BS], F32, tag="ps", name="r8_ps")
                nc.tensor.matmul(r8_ps[:], ones18f[:], rden[:], start=True, stop=True)
                pstate["r8_ps"] = r8_ps
            elif step == 5:
                nc.vector.tensor_tensor(gn_sb[:], g_sb[:], pstate["r8_ps"][:],
                                        mybir.AluOpType.mult)

        # ---- main pass: out = x @ W'.T + z8 @ U'.T.  Each bc block is
        # self-contained: 16x4 W-matmuls with the U DoubleRow matmuls
        # appended at dc14/15 (PSUM accumulation order is free), then the
        # per-ni cast+DMA overlap the next block.  bc0 carries the gating
        # chain (spread 2 dc apart so each tiny matmul's cross-engine
        # producer hides under the W stream) and the lam/z8 production.
        pstate = {}
        gate_at = {0: 0, 2: 1, 4: 2, 6: 3, 8: 4, 10: 5}
        lam_at = {11: 0, 12: 1, 13: 2, 14: 3}

        def emit_u_mm(psums, bc, ni, jj):
            nc.tensor.matmul(psums[ni][:],
                             z8[jj][:, :, bc * P:(bc + 1) * P],
                             ut8[:, 2 * jj:2 * jj + 2, ni * 512:(ni + 1) * 512],
                             start=False, stop=(jj == 1), perf_mode=DR)

        def emit_block(bc):
            psums = [ps.tile([P, 512], F32, tag="ps", name=f"po{bc}_{i}")
                     for i in range(NN)]
            for dc in range(ND):
                lhs = xws[dc][:, bc * P:(bc + 1) * P]
                for ni in range(NN):
                    nc.tensor.matmul(psums[ni][:], lhs,
                                     xws[dc][:, BS + ni * 512:BS + (ni + 1) * 512],
                                     start=(dc == 0), stop=False)
                if bc == 0 and dc in gate_at:
                    emit_gate_mm(gate_at[dc], pstate)
                if bc == 0 and dc in lam_at:
                    emit_lam_z(lam_at[dc], pstate)
                if dc == ND - 2:
                    for ni in range(NN):
                        emit_u_mm(psums, bc, ni, 0)
            for ni in range(NN):
                emit_u_mm(psums, bc, ni, 1)
            o_sb = opool.tile([P, D], BF16, tag="o", name=f"o{bc}")
            for ni in range(NN):
                nc.vector.tensor_copy(o_sb[:, ni * 512:(ni + 1) * 512], psums[ni][:])
                nc.scalar.dma_start(
                    out=outd[bc * P:(bc + 1) * P, ni * 512:(ni + 1) * 512],
                    in_=o_sb[:, ni * 512:(ni + 1) * 512])

        for bc in range(NB):
            emit_block(bc)


def build_program():
    nc = bacc.Bacc("TRN2", target_bir_lowering=False, debug=False)
    c8d = nc.dram_tensor("c8", (P, ND, C8W), F8E4, kind="ExternalInput").ap()
    xwd = nc.dram_tensor("xw", (D, BS + D), BF16, kind="ExternalInput").ap()
    ut8d = nc.dram_tensor("ut8", (P, NK, D), F8E5, kind="ExternalInput").ap()
    lamd = nc.dram_tensor("lam", (E, K), BF16, kind="ExternalInput").ap()
    w2cd = nc.dram_tensor("w2c", (1, E), BF16, kind="ExternalInput").ap()
    nabd = nc.dram_tensor("nab", (1, 2), F32, kind="ExternalInput").ap()
    outd = nc.dram_tensor("out", (BS, D), BF16, kind="ExternalOutput").ap()

    with tile.TileContext(nc) as tc:
        _emit(tc, nc, c8d, xwd, ut8d, lamd, w2cd, nabd, outd)
    nc.compile()
    return nc


def _get_prog():
    global _PROG
    if _PROG is None:
        _PROG = build_program()
    return _PROG


def make_in_maps(x, W, U_k, V_k, lambda_k, v, router_W1, router_W2):
    bf = ml_dtypes.bfloat16
    f8e4 = ml_dtypes.float8_e4m3
    f8e5 = ml_dtypes.float8_e5m2
    x = np.asarray(x, dtype=np.float32)
    W = np.asarray(W, dtype=np.float32)
    U_k = np.asarray(U_k, dtype=np.float32)
    V_k = np.asarray(V_k, dtype=np.float32)
    lambda_k = np.asarray(lambda_k, dtype=np.float32)
    v = np.asarray(v, dtype=np.float32)
    router_W1 = np.asarray(router_W1, dtype=np.float32)
    router_W2 = np.asarray(router_W2, dtype=np.float32)

    scale = 1.0 + v                                       # (D,) per output row n
    wt = np.ascontiguousarray((W * scale[:, None]).T).astype(bf)     # (d, n)
    # ut8[p, kc, n] = (U*(1+v))[n, kc*128+p] in e5m2, natural scale
    ut = (U_k * scale[:, None]).T                                    # (k, n)
    ut8 = np.ascontiguousarray(
        ut.reshape(NK, P, D).transpose(1, 0, 2)).astype(f8e5)        # (P, NK, D)
    u1 = (U_k.astype(np.float64) @ router_W1.astype(np.float64)).astype(np.float32)
    lam = np.ascontiguousarray(lambda_k / 32.0).astype(bf)           # (E, K)
    w2 = router_W2.reshape(-1)
    w2c = np.ascontiguousarray(w2.reshape(1, E)).astype(bf)
    nab = np.array([[-w2.max(), -w2.min()]], dtype=np.float32)

    # c8[p, d2, :] = [ x[b, d2*128+p] | 32*V[k, d2*128+p] | u1[d2*128+p] pad ]
    v32 = (32.0 * V_k).T.reshape(ND, P, K).transpose(1, 0, 2)        # (P, ND, K)
    u1c = u1.reshape(ND, P).T[:, :, None]                            # (P, ND, 1)
    pad = np.zeros((P, ND, 15), dtype=np.float32)

    in_maps = []
    for c in range(N_CORES):
        xs = x[c * BS:(c + 1) * BS]                                  # (BS, D)
        xw = np.ascontiguousarray(
            np.concatenate([xs.T.astype(bf), wt], axis=1))           # (D, BS+D)
        x8 = xs.T.reshape(ND, P, BS).transpose(1, 0, 2)              # (P, ND, BS)
        c8 = np.ascontiguousarray(
            np.concatenate([x8, v32, u1c, pad], axis=2)).astype(f8e4)
        in_maps.append({"c8": c8, "xw": xw, "ut8": ut8,
                        "lam": lam, "w2c": w2c, "nab": nab})
    return in_maps


def run(in_maps, trace=False):
    nc = _get_prog()
    res = run_bass_kernel_spmd(nc, in_maps, core_ids=list(range(N_CORES)), trace=trace)
    out = np.concatenate(
        [res.results[c]["out"].astype(np.float32) for c in range(N_CORES)], axis=0)
    return out, res


def kernel(x, W, U_k, V_k, lambda_k, v, router_W1, router_W2):
    in_maps = make_in_maps(x, W, U_k, V_k, lambda_k, v, router_W1, router_W2)
    out, _ = run(in_maps, trace=False)
    return out
